# revision 1
# baseline (speedup 1.0000x reference)
"""Trainium2 Bass kernel for nn_CMoSModel (moe_routing).

Data-parallel over batch: bs=256 -> 32 per core on 8 cores. Each core runs an
identical program on its shard; params replicated.

Math (reference):
  xt = x.T(b,c,L); mean/std over L; xn = (xt-mean)/std
  conv = depthwise_conv1d(xn, k=16, stride=8) + conv_b       [b,c,63]
  gates = softmax(conv @ gate_w.T + gate_b)                   [b,c,8]
  top2 -> softmax(vals) -> scatter back dense                 [b,c,8]
  y = einsum('bcsn,mon->bcmos', xn.resh, map_w) + map_b; out = (y.combine)*std+mean

Kernel layout: rows r=(b,c) on partitions (16 tiles of 128 = 2 batches x 64 ch),
L=512 on free dim.  Expert matmuls: for each s0 in 0..15, accumulate over m in
PSUM:  out[r, o] += (g_m*std*xn)[r, 16n+s0] @ map_w[m].T  via PE with
transposed gate-scaled copies; a 9th matmul adds (g*std)@map_b + mean*ones.
"""

import os
import sys

import numpy as np

for p in ("/opt/trn_rl_repo", "/opt/pypackages"):
    if p not in sys.path:
        sys.path.insert(0, p)

BS = 256
SEQ = 512
PRED = 720
C = 64
SEG = 16
NM = 8
KSZ = 16
STRIDE = 8
CONV_DIM = 63
N_IN = 32
N_OUT = 45
NCORES = 8
BPC = BS // NCORES   # 32 batches per core
NT = BPC // 2        # 16 tiles, 2 batches each (128 rows of (b,c))

_CACHE = {}


def _build_program(mm_dt_name="bfloat16"):
    import concourse.bass as bass
    import concourse.tile as tile
    from concourse import bacc
    from concourse import mybir
    from concourse.masks import make_identity

    f32 = mybir.dt.float32
    mm_dt = getattr(mybir.dt, mm_dt_name)
    AL = mybir.AluOpType
    AF = mybir.ActivationFunctionType
    AX = mybir.AxisListType

    nc = bacc.Bacc(None, target_bir_lowering=False)
    x_d = nc.declare_dram_parameter("x", [BPC, SEQ, C], f32, isOutput=False)
    cw_d = nc.declare_dram_parameter("conv_w", [C, 1, KSZ], f32, isOutput=False)
    cb_d = nc.declare_dram_parameter("conv_b", [C], f32, isOutput=False)
    gw_d = nc.declare_dram_parameter("gate_w", [NM, CONV_DIM], f32, isOutput=False)
    gb_d = nc.declare_dram_parameter("gate_b", [NM], f32, isOutput=False)
    mw_d = nc.declare_dram_parameter("map_w", [NM, N_OUT, N_IN], f32, isOutput=False)
    mb_d = nc.declare_dram_parameter("map_b", [NM, N_OUT], f32, isOutput=False)
    out_d = nc.declare_dram_parameter("out", [BPC, PRED, C], f32, isOutput=True)

    with tile.TileContext(nc) as tc:
        with (
            tc.tile_pool(name="consts", bufs=1) as consts,
            tc.tile_pool(name="xin", bufs=2) as xin,
            tc.tile_pool(name="work", bufs=2) as work,
            tc.tile_pool(name="small", bufs=3) as small,
            tc.tile_pool(name="xg", bufs=2) as xgp,
            tc.tile_pool(name="xgt", bufs=2) as xgtp,
            tc.tile_pool(name="oc", bufs=2) as ocp,
            tc.tile_pool(name="xg0", bufs=2) as xg0p,
            tc.tile_pool(name="psmall", bufs=1, space="PSUM") as psmall,
            tc.tile_pool(name="ptp", bufs=2, space="PSUM") as ptp,
            tc.tile_pool(name="py", bufs=2, space="PSUM") as pyp,
            tc.tile_pool(name="pxio", bufs=1, space="PSUM") as pxio,
            tc.tile_pool(name="pox", bufs=1, space="PSUM") as poxp,
        ):
            # ---- constants ----
            zero_t = consts.tile([128, 1], f32)
            nc.gpsimd.memset(zero_t[:], 0.0)
            nc.const_aps.aps[(f32, 0.0)] = zero_t[:]

            ident_f = consts.tile([128, 128], f32)
            make_identity(nc, ident_f[:])
            ident_m = consts.tile([128, 128], mm_dt)
            make_identity(nc, ident_m[:])

            cw_t = consts.tile([128, KSZ], f32)   # conv_w per-channel, dup 2x
            nc.sync.dma_start(cw_t[0:64, :], cw_d[:, 0, :])
            nc.sync.dma_start(cw_t[64:128, :], cw_d[:, 0, :])
            cb_t = consts.tile([128, 1], f32)
            nc.sync.dma_start(cb_t[0:64, :], cb_d[:, None])
            nc.sync.dma_start(cb_t[64:128, :], cb_d[:, None])

            gwT = consts.tile([CONV_DIM, NM], f32)  # gate_w.T
            nc.sync.dma_start(gwT[:, :], gw_d[:].rearrange("m d -> d m"))
            gb_t = consts.tile([128, NM], f32)      # gate_b bcast over partitions
            nc.sync.dma_start(
                gb_t[:, :], gb_d[None, :].broadcast_to([128, NM])
            )

            # map_w as [n, (m,o)]
            wT_f = consts.tile([N_IN, NM * N_OUT], f32)
            nc.sync.dma_start(
                wT_f[:, :], mw_d[:].rearrange("m o n -> n (m o)")
            )
            wT = consts.tile([N_IN, NM * N_OUT], mm_dt)
            nc.vector.tensor_copy(wT[:], wT_f[:])

            mbp_f = consts.tile([NM + 1, N_OUT], f32)  # [map_b; ones]
            nc.vector.memset(mbp_f[:, :], 1.0)
            nc.gpsimd.dma_start(mbp_f[0:NM, :], mb_d[:, :])
            mbp = consts.tile([NM + 1, N_OUT], mm_dt)
            nc.vector.tensor_copy(mbp[:], mbp_f[:])

            inv_L = 1.0 / SEQ

            for t in range(NT):
                # ---- load 2 batches naturally, PE-transpose to [c, L] ----
                xt = xin.tile([128, SEQ], f32, tag="xt")
                for h in range(2):
                    xraw = xin.tile([128, 4 * C], f32, tag="xraw")
                    xrv = xraw[:].rearrange("p (j c) -> p j c", j=4)
                    nc.sync.dma_start(
                        xrv, x_d[2 * t + h].rearrange("(j p) c -> p j c", p=128)
                    )
                    psx = pxio.tile([64, SEQ], f32, tag="pxio")
                    for j in range(4):
                        nc.tensor.transpose(
                            psx[:, j * 128 : (j + 1) * 128], xrv[:, j], ident_f[:]
                        )
                    if h == 0:
                        nc.vector.tensor_copy(xt[0:64, :], psx[:])
                    else:
                        nc.scalar.copy(xt[64:128, :], psx[:])

                # ---- stats ----
                s1 = small.tile([128, 1], f32, tag="s1")
                nc.vector.tensor_reduce(s1[:], xt[:], axis=AX.X, op=AL.add)
                mean = small.tile([128, 1], f32, tag="mean")
                nc.vector.tensor_scalar(mean[:], s1[:], inv_L, None, AL.mult)
                xc = work.tile([128, SEQ], f32, tag="xc")
                mc = small.tile([128, 1], f32, tag="mc")
                nc.vector.tensor_scalar(
                    xc[:], xt[:], mean[:], None, AL.subtract, AL.add,
                    accum_out=mc[:],
                )
                sq = work.tile([128, SEQ], f32, tag="sq")
                m2 = small.tile([128, 1], f32, tag="m2")
                nc.scalar.activation(sq[:], xc[:], AF.Square, accum_out=m2[:])
                # var = m2/L - (mc/L)^2 ; std = sqrt(var + 1e-10); rstd = 1/std
                mcn = small.tile([128, 1], f32, tag="mcn")
                nc.vector.tensor_scalar(mcn[:], mc[:], inv_L, None, AL.mult)
                var = small.tile([128, 1], f32, tag="var")
                nc.vector.scalar_tensor_tensor(
                    var[:], mcn[:], -1.0, mcn[:], AL.mult, AL.mult
                )  # -mcn^2
                nc.vector.scalar_tensor_tensor(
                    var[:], m2[:], inv_L, var[:], AL.mult, AL.add
                )
                nc.vector.tensor_scalar(var[:], var[:], 1e-10, None, AL.add)
                std = small.tile([128, 1], f32, tag="std")
                nc.scalar.activation(std[:], var[:], AF.Sqrt)
                rstd = small.tile([128, 1], f32, tag="rstd")
                nc.vector.reciprocal(rstd[:], std[:])

                xn = work.tile([128, SEQ], f32, tag="xn")
                nc.scalar.mul(xn[:], xc[:], rstd[:])

                # ---- depthwise conv (gpsimd), conv[d] = sum_k xn[8d+k]*w[k] ----
                cv = small.tile([128, CONV_DIM], f32, tag="cv")
                xnv = xn[:].rearrange("p (d k) -> p d k", k=STRIDE)  # [128,64,8]
                for k in range(KSZ):
                    src = xnv[:, (k // STRIDE) : (k // STRIDE) + CONV_DIM, k % STRIDE]
                    if k == 0:
                        nc.vector.tensor_scalar(
                            cv[:], src, cw_t[:, 0:1], None, AL.mult
                        )
                    else:
                        nc.vector.scalar_tensor_tensor(
                            cv[:], src, cw_t[:, k : k + 1], cv[:], AL.mult, AL.add
                        )
                nc.vector.tensor_scalar(cv[:], cv[:], cb_t[:], None, AL.add)

                # ---- gate logits via PE: transpose conv, then [63,128]^T@[63,8] ----
                cps = psmall.tile([CONV_DIM, 128], f32, tag="ps")
                nc.tensor.transpose(cps[:], cv[:], ident_f[:])
                cvT = small.tile([CONV_DIM, 128], f32, tag="cvT")
                nc.vector.tensor_copy(cvT[:], cps[:])
                lps = psmall.tile([128, NM], f32, tag="ps")
                nc.tensor.matmul(lps[:], cvT[:], gwT[:], start=True, stop=True)
                lg = small.tile([128, NM], f32, tag="lg")
                nc.vector.tensor_tensor(lg[:], lps[:], gb_t[:], AL.add)

                # ---- softmax over 8, top-2, renorm softmax, dense gates ----
                E1 = small.tile([128, NM], f32, tag="E1")
                se1 = small.tile([128, 1], f32, tag="se1")
                nc.scalar.activation(E1[:], lg[:], AF.Exp, accum_out=se1[:])
                r1 = small.tile([128, 1], f32, tag="r1")
                nc.vector.reciprocal(r1[:], se1[:])
                v = small.tile([128, NM], f32, tag="v")
                nc.vector.tensor_scalar(v[:], E1[:], r1[:], None, AL.mult)
                E2 = small.tile([128, NM], f32, tag="E2")
                nc.scalar.activation(E2[:], v[:], AF.Exp)
                m8 = small.tile([128, 8], f32, tag="m8")
                nc.vector.max(m8[:], E2[:])
                msk = small.tile([128, NM], f32, tag="msk")
                nc.vector.tensor_scalar(msk[:], E2[:], m8[:, 1:2], None, AL.is_ge)
                Em = small.tile([128, NM], f32, tag="Em")
                se2 = small.tile([128, 1], f32, tag="se2")
                nc.vector.scalar_tensor_tensor(
                    Em[:], E2[:], 1.0, msk[:], AL.bypass, AL.mult, accum_out=se2[:]
                )
                r2 = small.tile([128, 1], f32, tag="r2")
                nc.vector.reciprocal(r2[:], se2[:])
                g = small.tile([128, NM], f32, tag="g")
                nc.vector.tensor_scalar(g[:], Em[:], r2[:], None, AL.mult)

                # gsm = [g*std, mean] -> transposed for the correction matmul
                gsm = small.tile([128, NM + 1], f32, tag="gsm")
                nc.vector.tensor_scalar(gsm[:, 0:NM], g[:], std[:], None, AL.mult)
                nc.vector.tensor_copy(gsm[:, NM : NM + 1], mean[:])
                gps = psmall.tile([NM + 1, 128], f32, tag="ps")
                nc.tensor.transpose(gps[:], gsm[:], ident_f[:])
                gsmT = small.tile([NM + 1, 128], mm_dt, tag="gsmT")
                nc.vector.tensor_copy(gsmT[:], gps[:])

                # ---- gate*std-scaled copies of xn, cols pre-permuted so each
                # s-quad transpose block is a contiguous 128-col slice:
                # xg col (m, q, s, n) = m*512 + q*128 + s*32 + n  <- xn[16n+4q+s]
                xg = xgp.tile([128, NM * SEQ], mm_dt, tag="xg")
                xgv = xg[:].rearrange(
                    "p (m q s n) -> p m n q s", m=NM, q=4, s=4, n=N_IN
                )
                xnv2 = xn[:].rearrange("p (n q s) -> p n q s", n=N_IN, q=4, s=4)
                for m in range(NM):
                    sc = gsm[:, m : m + 1]
                    if m % 2 == 0:
                        nc.vector.tensor_scalar(xgv[:, m], xnv2, sc, None, AL.mult)
                    else:
                        nc.scalar.mul(xgv[:, m], xnv2, sc)

                # ---- PE transposes: per m, 4 s-quad blocks -> [s*32+n, r] ----
                xgt = xgtp.tile([128, NM * SEQ], mm_dt, tag="xgt")
                for m in range(NM):
                    tp = ptp.tile([128, SEQ], mm_dt, tag="tp")
                    for q in range(4):
                        nc.tensor.transpose(
                            tp[:, q * 128 : (q + 1) * 128],
                            xg[:, m * SEQ + q * 128 : m * SEQ + (q + 1) * 128],
                            ident_m[:],
                        )
                    dst = xgt[:, m * SEQ : (m + 1) * SEQ]
                    if m % 2 == 0:
                        nc.vector.tensor_copy(dst, tp[:])
                    else:
                        nc.scalar.copy(dst, tp[:])

                # ---- rearrange strips to partition-base 0 (PE can't switch
                # row strips within a kernel on this toolchain) ----
                xg0 = xg0p.tile([N_IN, 4 * NM * SEQ], mm_dt, tag="xg0")
                xg0v = xg0[:].rearrange(
                    "p (m q s r) -> p m q s r", m=NM, q=4, s=4, r=128
                )
                xgtv = xgt[:].rearrange("p (m q r) -> p m q r", m=NM, q=4)
                for spp in range(4):
                    nc.sync.dma_start(
                        xg0v[:, :, :, spp],
                        xgtv[32 * spp : 32 * (spp + 1)],
                    )

                # ---- expert matmuls, accumulated over m in PSUM + correction ----
                oc = ocp.tile([128, PRED], f32, tag="oc")
                ocv = oc[:].rearrange("p (o s) -> p o s", o=N_OUT, s=SEG)
                for half in range(2):
                    yp = pyp.tile([128, 8 * N_OUT], f32, tag="yp")
                    for s0h in range(8):
                        s0 = half * 8 + s0h
                        q, sp = s0 // 4, s0 % 4
                        dst = yp[:, s0h * N_OUT : (s0h + 1) * N_OUT]
                        for m in range(NM):
                            nc.tensor.matmul(
                                dst,
                                xg0v[:, m, q, sp],
                                wT[:, m * N_OUT : (m + 1) * N_OUT],
                                start=(m == 0),
                                stop=False,
                            )
                        nc.tensor.matmul(
                            dst, gsmT[:], mbp[:], start=False, stop=True
                        )
                    ypv = yp[:].rearrange("p (s o) -> p o s", s=8, o=N_OUT)
                    dstv = ocv[:, :, half * 8 : (half + 1) * 8]
                    if half == 0:
                        nc.vector.tensor_copy(dstv, ypv)
                    else:
                        nc.scalar.copy(dstv, ypv)

                # ---- store: PE-transpose [128,120]-blocks -> [120,128] ----
                ocv6 = oc[:].rearrange("p (i u) -> p i u", i=6)  # u=120
                pox = poxp.tile([120, 6 * 128], f32, tag="pox")
                poxv = pox[:].rearrange("p (i r) -> p i r", i=6)
                for i in range(6):
                    nc.tensor.transpose(poxv[:, i], ocv6[:, i], ident_f[:])
                ocs = ocp.tile([120, 6 * 128], f32, tag="ocs")
                nc.vector.tensor_copy(ocs[0:64, :], pox[0:64, :])
                nc.scalar.copy(ocs[64:120, :], pox[64:120, :])
                ocsv = ocs[:].rearrange("p (i h c) -> p i h c", i=6, h=2)
                for h in range(2):
                    nc.sync.dma_start(
                        out_d[2 * t + h].rearrange("(i p) c -> p i c", p=120),
                        ocsv[:, :, h],
                    )

    nc.compile()
    return nc


def _get_program(mm_dt_name):
    key = mm_dt_name
    if key not in _CACHE:
        _CACHE[key] = _build_program(key)
    return _CACHE[key]


def kernel(x, conv_w, conv_b, gate_w, gate_b, map_w, map_b, _mm_dt="bfloat16",
           _trace=False):
    from concourse.bass_utils import run_bass_kernel_spmd

    nc = _get_program(_mm_dt)
    x = np.ascontiguousarray(np.asarray(x, dtype=np.float32))
    params = dict(
        conv_w=np.ascontiguousarray(np.asarray(conv_w, np.float32)),
        conv_b=np.ascontiguousarray(np.asarray(conv_b, np.float32)),
        gate_w=np.ascontiguousarray(np.asarray(gate_w, np.float32)),
        gate_b=np.ascontiguousarray(np.asarray(gate_b, np.float32)),
        map_w=np.ascontiguousarray(np.asarray(map_w, np.float32)),
        map_b=np.ascontiguousarray(np.asarray(map_b, np.float32)),
    )
    in_maps = [
        dict(x=x[i * BPC : (i + 1) * BPC], **params) for i in range(NCORES)
    ]
    res = run_bass_kernel_spmd(
        nc, in_maps, core_ids=list(range(NCORES)), trace=_trace
    )
    out = np.concatenate([res.results[i]["out"] for i in range(NCORES)], axis=0)
    if _trace:
        return out, res
    return out

